# revision 69
# baseline (speedup 1.0000x reference)
"""Trainium2 Bass kernel: single-head causal self-attention.

Problem: B=4, S=2048, D=1024, f32 IO.
  Q = x@Wq + bq; K = x@Wk + bk; V = x@Wv + bv
  out = softmax(causal(Q K^T / sqrt(D))) @ V

Sharding over 8 NeuronCores: core c -> (batch b = c//2, part p = c%2).
Each batch's 2048 query rows are split into eight 256-row windows
W_0..W_7 (causal depth of W_m is 2m+2 k-tiles of 128).  p=0 takes
windows [0,3,4,7], p=1 takes [1,2,5,6] -- both depth-sum to 36 so
attention work is balanced.  The SPMD graph is identical on every core:
4 window "slots" with fixed half-depths Dj=[2,4,6,8]; which global
window sits in which slot is data placement (host permutes Q rows and
output rows; boundary causal masks are inputs).

K/V are PARITY-SPLIT across the core pair: core p projects K^T and V
for global k-tiles {2j+p} only ("own", canonical positions).  The pair
exchanges halves with AllGathers -- K^T in fp8-e4m3 (2x512KB, only the
PEER half of the scores sees fp8; measured rel-err 1.266e-2 vs the
2e-2 gate), V in bf16 (2x1MB) -- and each core reconstructs the peer
half as peer = member0 + member1 - own, which is SPMD-uniform.  For
fp8 the m0+m1 sum is formed in a bf16 temp (exact for e4m3 addends) so
subtracting the fp8 own payload returns the peer member bit-exactly.

Attention is two passes: ALL own-parity phases first (slot partial O
and exp-sums evicted to SBUF via ACT, sums first -- psS bank reuse),
then all peer-parity phases combined with the own partials on DVE and
normalized+biased via one scalar_tensor_tensor (o*[1/sum] + bv; the
V-bias is NEVER projected -- normalization absorbs the sum*bv term
exactly).  Slot 0 runs last with own+peer fused in one PSUM
accumulation so the exit path is normalize-only.

Hardware model, learned the hard way (timings at ~2.0-2.2GHz P0):
- The SERIAL collective stream is the pass-2 critical path: it cannot
  start before the framework's rendezvous barrier clears (~54us) plus
  ~11us first-AG ncfw startup, runs ~12-24us/MB-per-rank (2-rank mesh,
  noisy -- 4 pairs contend), and its SDMA traffic starves DIRECT2D
  reads on all 16 rings while it runs.  Bytes are everything.
- DIRECT2D (engine-triggered) DMA costs ~200-300ns PER PARTITION-ROW
  DESCRIPTOR regardless of size, so every [128, N] bounce transfer is
  ~25us on one ring.  All bounce traffic moves as [32..64, N]
  partition strips on separate rings (strip_writes/strip_reads).
- AllGather triggers MUST be on gpsimd (walrus: DMA|Pool only).  Keep
  the gpsimd queue free of Q7 COMPUTE ops before/between triggers:
  partition_broadcast or tensor_tensor there stalls the strict FIFO
  (+20-30us).  bvB broadcast is a K=1 ones-row matmul instead.
- Engine-queue FIFO order must match semaphore-readiness order per
  queue (sync: inputs, bounce writes, bounce reads, out stores; ACT
  never carries anything that waits on a collective).
- fp8 DVE TTs get no 16-bit fast path (~2.3us per [P,2048]); keep all
  merge TTs on DVE anyway -- Q7 TTs are slower AND block the gpsimd
  read queue.
- PE runs at ~2.0-2.2GHz under all-core load (P0), ~232ns/512-col MM.

- The first krecv chunk's reads are PRE-ISSUED right after the AG-V1
  trigger so they fire the moment AG-K1 lands, inside the clean ring
  window before AG-V1's SDMA traffic starts -- emitted at their
  natural position they could only fire ~7us later and the whole
  merge chain slipped behind pass-2's deadlines.
- Input chunks narrower than ~2 d-tiles are a LOSS (measured +35us):
  per-partition descriptor granularity applies to input DMAs too, so
  [P, 1-di] chunks halve effective arrival bandwidth.

Matmuls bf16 (peer-half scores fp8xbf16), softmax f32, partial-O
evictions bf16.  HW exec ~194-200us (collective-stream noise).
"""

import sys

import numpy as np

if "/opt/trn_rl_repo" not in sys.path:
    sys.path.insert(0, "/opt/trn_rl_repo")

import ml_dtypes

D = 1024
S = 2048
B = 4
P = 128
NCORES = 8
WINDOWS = {0: [0, 3, 4, 7], 1: [1, 2, 5, 6]}
DDEPTH = [2, 4, 6, 8]  # per-slot half-depth (own tiles = peer tiles = Dj)
BF16 = ml_dtypes.bfloat16

_GRAPH = None


def _build_graph():
    import concourse.bass as bass  # noqa: F401
    from concourse import bacc, mybir, tile

    f32 = mybir.dt.float32
    b16 = mybir.dt.bfloat16

    nc = bacc.Bacc(None, target_bir_lowering=False, debug=False, num_devices=NCORES)

    # Host-pre-arranged partition-major images (contiguous per partition).
    # xt: x^T columns of OWN parity k-tiles, [128, 2 s-half, 8 d_in, 512]
    # w*: [128, 2 out-half, 8 d_in, 512]
    xt_d = nc.declare_dram_parameter("xt", [P, 8192], b16, False)
    xqt_d = nc.declare_dram_parameter("xqt", [P, 8192], b16, False)
    wq_d = nc.declare_dram_parameter("wq", [P, 8192], b16, False)
    wk_d = nc.declare_dram_parameter("wk", [P, 8192], b16, False)
    wv_d = nc.declare_dram_parameter("wv", [P, 8192], b16, False)
    bq_d = nc.declare_dram_parameter("bq", [P, 8], f32, False)
    bk_d = nc.declare_dram_parameter("bk", [P, 8], f32, False)
    bvb_d = nc.declare_dram_parameter("bvb", [1, D], b16, False)
    mk_d = nc.declare_dram_parameter("masks", [P, 8 * 256], b16, False)
    id_d = nc.declare_dram_parameter("ident", [P, P], b16, False)
    out_d = nc.declare_dram_parameter("out", [1024, D], f32, True)

    f8 = mybir.dt.float8e4

    # Collective bounce buffers (internal DRAM).  The serial collective
    # stream (one ncfw; AGs never overlap) is the pass-2 critical path:
    # it can't start before the framework's rendezvous barrier clears
    # (~55us) and runs at ~17us/MB, so bytes are everything.  K^T is
    # exchanged in fp8-e4m3 (scores for the peer half only; measured
    # rel-err 1.27e-2 vs the 2e-2 gate) as TWO 512KB AllGathers -- the
    # first fires mid-K-projection.  V goes as two 1MB bf16 AGs.
    ksend1 = nc.dram_tensor("ksend1", [P, 4 * 1024], f8)
    krecv1 = nc.dram_tensor("krecv1", [2, P, 4 * 1024], f8)
    ksend2 = nc.dram_tensor("ksend2", [P, 4 * 1024], f8)
    krecv2 = nc.dram_tensor("krecv2", [2, P, 4 * 1024], f8)
    vsend1 = nc.dram_tensor("vsend1", [P, 4 * 1024], b16)
    vrecv1 = nc.dram_tensor("vrecv1", [2, P, 4 * 1024], b16)
    vsend2 = nc.dram_tensor("vsend2", [P, 4 * 1024], b16)
    vrecv2 = nc.dram_tensor("vrecv2", [2, P, 4 * 1024], b16)
    groups = [[0, 1], [2, 3], [4, 5], [6, 7]]

    with tile.TileContext(nc) as tc:
        with (
            tc.tile_pool(name="const", bufs=1) as const,
            tc.tile_pool(name="psA", bufs=2, space="PSUM") as psA,
            tc.tile_pool(name="psB", bufs=2, space="PSUM") as psB,
            tc.tile_pool(name="psS", bufs=2, space="PSUM") as psS,
            tc.tile_pool(name="evict", bufs=6) as evict,
            tc.tile_pool(name="evsm", bufs=2) as evsm,
            tc.tile_pool(name="evss", bufs=4) as evss,
            tc.tile_pool(name="mtmp", bufs=2) as mtmp,
            tc.tile_pool(name="ktmp8", bufs=2) as ktmp8,
        ):
            xt = const.tile([P, 2, 8, 512], b16, name="xt_s")
            xqt = const.tile([P, 2, 8, 512], b16, name="xqt_s")
            # wq/wk are et-major ([P, d_out tile, d_in tile, 128]) so the K
            # projection's first chunk needs only a 256KB weight slice; wv
            # stays half-major (its matmuls stream full 512-wide slices).
            w_sb = {
                n: const.tile(
                    [P, 8, 8, P] if n in ("q", "k") else [P, 2, 8, 512],
                    b16,
                    name=f"w_{n}_s",
                )
                for n in ("q", "k", "v")
            }
            kT_own = const.tile([P, 8, 1024], b16, name="kT_own")
            # fp8 shadow of kT_own: what this core SENDS (and what the
            # SPMD merge peer = m0 + m1 - own must subtract -- it has to
            # be bit-identical to the member payload).  Pass-1 keeps the
            # bf16 kT_own; only pass-2 sees fp8 K.
            kT_own8 = const.tile([P, 8, 1024], f8, name="kT_own8")
            kT_peer = const.tile([P, 8, 1024], f8, name="kT_peer")
            vv = const.tile([P, 16, D], b16, name="vv_s")
            qT = const.tile([P, 8, 1024], b16, name="qT_s")
            bq_s = const.tile([P, 8], f32, name="bq_s")
            bk_s = const.tile([P, 8], f32, name="bk_s")
            bvB = const.tile([P, D], b16, name="bvB_s")
            bvr_s = bvB[0:1, :]  # bv row loads into partition 0 of its own broadcast tile
            # Mask patterns depend only on slot parity (m - Dj alternates
            # with slot%2 per core), so 8 entries cover all 16 uses.
            mk_s = const.tile([P, 8, 256], b16, name="mk_s")
            ones_s = const.tile([P, 1], b16, name="ones_s")
            id_s = const.tile([P, P], b16, name="id_s")
            oOwn = const.tile([P, 8, 1024], b16, name="oOwn_s")
            sums_own = const.tile([P, 8], f32, name="sums_own")

            # Input DMAs: ordered so K projection unblocks first -- the
            # first chunk needs only bk + wk[et 0] (256KB) + xt half 0.
            # The K-proj-critical tensors are split across several
            # dma_starts (each lands on its own DMA ring, ~40GB/s/ring)
            # and the xt chunks issue from the scalar queue so the two
            # sequencers enqueue descriptors in parallel.
            nc.sync.dma_start(bk_s[:], bk_d.ap())
            nc.sync.dma_start(w_sb["k"][:, 0, 0:4], wk_d.ap()[:, 0:512])
            nc.sync.dma_start(w_sb["k"][:, 0, 4:8], wk_d.ap()[:, 512:1024])
            for d2 in range(4):
                nc.scalar.dma_start(
                    xt[:, 0, 2 * d2 : 2 * d2 + 2],
                    xt_d.ap()[:, d2 * 1024 : (d2 + 1) * 1024],
                )
            for d2 in range(4):
                nc.sync.dma_start(
                    xt[:, 1, 2 * d2 : 2 * d2 + 2],
                    xt_d.ap()[:, 4096 + d2 * 1024 : 4096 + (d2 + 1) * 1024],
                )
            for et in range(1, 8):
                nc.sync.dma_start(
                    w_sb["k"][:, et], wk_d.ap()[:, et * 1024 : (et + 1) * 1024]
                )
            nc.sync.dma_start(bvr_s, bvb_d.ap())
            for c in range(2):
                nc.sync.dma_start(
                    w_sb["v"][:, c], wv_d.ap()[:, c * 4096 : (c + 1) * 4096]
                )
            nc.sync.dma_start(bq_s[:], bq_d.ap())
            for et in range(8):
                nc.sync.dma_start(
                    w_sb["q"][:, et], wq_d.ap()[:, et * 1024 : (et + 1) * 1024]
                )
            for c in range(2):
                nc.sync.dma_start(xqt[:, c], xqt_d.ap()[:, c * 4096 : (c + 1) * 4096])
            nc.sync.dma_start(mk_s[:], mk_d.ap().rearrange("p (n f) -> p n f", f=256))
            nc.sync.dma_start(id_s[:], id_d.ap())
            nc.any.memset(ones_s[:], 1.0)

            # PE warm-up: junk matmuls during the initial DMA wait flip the
            # HAM clock gate before the real work starts (first ~3.4us of
            # matmul activity otherwise runs at 1.2GHz).  Sized so the
            # queue drains right as the split xt/wk chunks land (~12us);
            # extra warm-ups would delay K-proj in the in-order PE FIFO.
            warm_in = const.tile([P, 512], b16, name="warm_in")
            nc.vector.memset(warm_in[:], 0.0)
            # Tiny TT warms the Q7 tensor-op ucode library so the first
            # real gpsimd merge op doesn't pay a LIBRARY_RELOAD mid-
            # exchange (the reload itself is cheap, but it must happen
            # before the strict-FIFO queue fills with gated DMAs).
            nc.gpsimd.tensor_tensor(
                warm_in[:, 0:1], ones_s[:], ones_s[:], mybir.AluOpType.add
            )
            for _ in range(2):
                wps = psA.tile([P, 512], f32, name="ps_mm")
                for _ in range(4):
                    nc.tensor.matmul(wps[:1], ones_s[:], warm_in[:], start=True, stop=True)

            ident = mybir.ActivationFunctionType.Identity
            ones_r = const.tile([1, P], b16, name="ones_r")
            nc.any.memset(ones_r[:], 1.0)

            def strip_writes(dst_dram, src_fn, queues):
                """Bounce-buffer write as 4 partition strips.  A DIRECT2D
                descriptor covers ONE partition's contiguous run (~185ns
                each regardless of size), so a [128, N] write costs ~24us
                on one ring no matter what N is.  [32, N] strips on 4
                rings move the same data ~4x faster."""
                for s in range(4):
                    queues[s % len(queues)].dma_start(
                        dst_dram.ap()[32 * s : 32 * (s + 1), :],
                        src_fn(32 * s, 32 * (s + 1)),
                    )

            def strip_reads(dst_fn, src_ap, queues, nstrip=2):
                """Post-collective DRAM->SBUF reads as partition strips
                (same per-partition-descriptor economics as the writes)."""
                step = P // nstrip
                for s in range(nstrip):
                    queues[s % len(queues)].dma_start(
                        dst_fn(step * s, step * (s + 1)),
                        src_ap[step * s : step * (s + 1)],
                    )

            # K^T projection (own-parity 1024 keys):
            # psum[d_out 128, s 512] = sum_di Wk[di,e].T @ xT[di,s]
            for et in range(8):
                ksend = ksend1 if et < 4 else ksend2
                for sw in range(2):
                    ps = psA.tile([P, 512], f32, name="ps_mm")
                    for di in range(8):
                        nc.tensor.matmul(
                            ps[:],
                            w_sb["k"][:, et, di, :],
                            xt[:, sw, di, :],
                            start=(di == 0),
                            stop=(di == 7),
                        )
                    nc.scalar.activation(
                        kT_own[:, et, sw * 512 : (sw + 1) * 512],
                        ps[:],
                        ident,
                        bias=bk_s[:, et : et + 1],
                        scale=1.0,
                    )
                    nc.scalar.activation(
                        kT_own8[:, et, sw * 512 : (sw + 1) * 512],
                        ps[:],
                        ident,
                        bias=bk_s[:, et : et + 1],
                        scale=1.0,
                    )
                if et == 3 or et == 7:
                    strip_writes(
                        ksend,
                        lambda a, b, lo=et - 3: kT_own8[a:b, lo : lo + 4, :],
                        [nc.sync, nc.scalar, nc.gpsimd, nc.sync],
                    )
                    nc.gpsimd.collective_compute(
                        "AllGather",
                        mybir.AluOpType.bypass,
                        replica_groups=groups,
                        ins=[(ksend1 if et == 3 else ksend2).ap().opt()],
                        outs=[(krecv1 if et == 3 else krecv2).ap().opt()],
                    )

            # V projection: OWN-parity 8 k-tiles only; the peer half
            # arrives via a single 2MB AllGather (collective cost is
            # roughly per-op, not per-byte, at these sizes).
            for st in range(8):
                for dw in range(2):
                    ps = psA.tile([P, 512], f32, name="ps_mm")
                    for di in range(8):
                        nc.tensor.matmul(
                            ps[:],
                            xt[:, st // 4, di, (st % 4) * P : (st % 4 + 1) * P],
                            w_sb["v"][:, dw, di, :],
                            start=(di == 0),
                            stop=(di == 7),
                        )
                    nc.scalar.activation(
                        vv[:, st, dw * 512 : (dw + 1) * 512], ps[:], ident, scale=1.0
                    )
                if st in (3, 7):
                    vsend, vrecv, lo = {
                        3: (vsend1, vrecv1, 0),
                        7: (vsend2, vrecv2, 4),
                    }[st]
                    strip_writes(
                        vsend,
                        lambda a, b, lo=lo: vv[a:b, lo : lo + 4, :],
                        [nc.sync, nc.scalar, nc.gpsimd, nc.sync],
                    )
                    nc.gpsimd.collective_compute(
                        "AllGather",
                        mybir.AluOpType.bypass,
                        replica_groups=groups,
                        ins=[vsend.ap().opt()],
                        outs=[vrecv.ap().opt()],
                    )
                if st == 3:
                    # Pre-issue the first krecv chunk's reads HERE (right
                    # after the AG-V1 trigger in the sync/gpsimd FIFOs):
                    # emitted any later they queue behind the vsend2
                    # writes and can only fire ~86us; from here they fire
                    # the moment AG-K1 lands (~79us), inside the clean
                    # ring window before AG-V1's SDMA traffic starts.
                    ek8 = ktmp8.tile([P, 2, 1024], f8, name="merge_f8")
                    strip_reads(
                        lambda a, b: ek8[a:b],
                        krecv1.ap()[0, :, 0:2048].rearrange(
                            "p (o s) -> p o s", s=1024
                        ),
                        [nc.gpsimd, nc.sync],
                    )
                    ekt = ktmp8.tile([P, 2, 1024], f8, name="merge_f8")
                    strip_reads(
                        lambda a, b: ekt[a:b],
                        krecv1.ap()[1, :, 0:2048].rearrange(
                            "p (o s) -> p o s", s=1024
                        ),
                        [nc.sync, nc.gpsimd],
                    )

            # Q^T projection (this core's 1024 query rows)
            for et in range(8):
                for sw in range(2):
                    ps = psA.tile([P, 512], f32, name="ps_mm")
                    for di in range(8):
                        nc.tensor.matmul(
                            ps[:],
                            w_sb["q"][:, et, di, :],
                            xqt[:, sw, di, :],
                            start=(di == 0),
                            stop=(di == 7),
                        )
                    nc.scalar.activation(
                        qT[:, et, sw * 512 : (sw + 1) * 512],
                        ps[:],
                        ident,
                        bias=bq_s[:, et : et + 1],
                        scale=1.0,
                    )

            # V-bias never rides the projection: out = (eS @ V0)/sum + bv
            # exactly (the normalization absorbs the sum*bv term), so bv
            # is broadcast once to all partitions here (K=1 ones-row
            # matmul -- NOT gpsimd.partition_broadcast, whose Q7 ucode
            # LIBRARY_RELOAD would block the gpsimd queue, and with it
            # the AllGather triggers, for ~30us; emitted after the
            # projections so it never delays them in the in-order PE
            # FIFO) and added at the normalize step.
            for half in range(2):
                ps = psA.tile([P, 512], f32, name="ps_mm")
                nc.tensor.matmul(
                    ps[:],
                    ones_r[0:1, :],
                    bvr_s[:, half * 512 : (half + 1) * 512],
                    start=True,
                    stop=True,
                )
                nc.scalar.activation(
                    bvB[:, half * 512 : (half + 1) * 512], ps[:], ident, scale=1.0
                )

            # K merge on DVE: peer = member0 + member1 - own (SPMD-uniform).
            # The DVE queue has no other work between the projections and
            # the pass-2 combines, so a late AllGather can only delay the
            # merge itself, never an unrelated op queued behind it.
            # Reads issue from gpsimd and sync AFTER their bounce-write
            # strips (queue order matches semaphore readiness order, so
            # no FIFO inversion can strand a later-ready transfer).
            # fp8 wrinkle: m0 + m1 must be formed in a bf16 temp (exact
            # for e4m3 addends); subtracting the fp8 own payload then
            # yields the peer member's exact e4m3 value, so the final fp8
            # store is lossless.
            for h, krecv in enumerate((krecv1, krecv2)):
                for c in range(2):
                    o0 = 4 * h + 2 * c
                    if h == 0 and c == 0:
                        k8, kt = ek8, ekt  # pre-issued at st==3 above
                    else:
                        k8 = ktmp8.tile([P, 2, 1024], f8, name="merge_f8")
                        strip_reads(
                            lambda a, b, k8=k8: k8[a:b],
                            krecv.ap()[0, :, c * 2048 : (c + 1) * 2048].rearrange(
                                "p (o s) -> p o s", s=1024
                            ),
                            [nc.gpsimd, nc.sync],
                        )
                        kt = ktmp8.tile([P, 2, 1024], f8, name="merge_f8")
                        strip_reads(
                            lambda a, b, kt=kt: kt[a:b],
                            krecv.ap()[1, :, c * 2048 : (c + 1) * 2048].rearrange(
                                "p (o s) -> p o s", s=1024
                            ),
                            [nc.sync, nc.gpsimd],
                        )
                    # fp8 TTs get no 16-bit fast path (~2.3us per op).
                    # Both halves stay on DVE: Q7 tensor ops measured even
                    # slower AND strict-FIFO-block the v_merge strip reads
                    # queued behind them on gpsimd (+29us end-to-end).
                    eng = nc.vector
                    tb = mtmp.tile([P, 2, 1024], b16, name="merge_tmp")
                    eng.tensor_tensor(tb[:], k8[:], kt[:], mybir.AluOpType.add)
                    eng.tensor_tensor(
                        kT_peer[:, o0 : o0 + 2, :],
                        tb[:],
                        kT_own8[:, o0 : o0 + 2, :],
                        mybir.AluOpType.subtract,
                    )

            # Attention pass 1: OWN-parity k-tiles for every slot (deepest
            # first).  scores^T [k 128, q 256] per tile; exp on ACT (f32
            # psum -> bf16); boundary masks on GPSIMD for the last 2 tiles;
            # O_unnorm[q,d] += expS^T.T @ V; sums via ones-matmul.  Partial
            # O and sums evict to SBUF (ACT) to free PSUM for the next slot.
            inv_sqrt_d = float(1.0 / np.sqrt(D))
            exp_f = mybir.ActivationFunctionType.Exp

            def score_tile(kTt, slot, i, Dj, phase):
                """scores^T psum for k-tile i of a slot's phase.  Boundary
                tiles get the additive -1e30 mask as a 9th accumulation
                matmul (identity lhsT) -- keeps the mask on the PE so the
                exp -> AV chain never waits on another engine."""
                q0 = 256 * slot
                masked = i >= Dj - 2
                ps = psA.tile([P, 256], f32, name="ps_mm")
                for di in range(8):
                    nc.tensor.matmul(
                        ps[:, :256],
                        kTt[:, di, i * P : (i + 1) * P],
                        qT[:, di, q0 : q0 + 256],
                        start=(di == 0),
                        stop=(di == 7) and not masked,
                    )
                if masked:
                    nc.tensor.matmul(
                        ps[:, :256],
                        id_s[:],
                        mk_s[:, (slot % 2) * 4 + phase * 2 + (i - (Dj - 2)), :],
                        start=False,
                        stop=True,
                    )
                eS = evict.tile([P, 256], b16, name="eS")
                nc.scalar.activation(eS[:], ps[:, :256], exp_f, scale=inv_sqrt_d)
                return eS

            def av_tile(eS, pO, pSm, vti, first, last):
                # pO[jj] is a 2-bank [P, 1024] psum tile; each matmul still
                # targets a single bank via the half slice.
                for jj in range(2):
                    st_op = eS[:, jj * P : (jj + 1) * P]
                    nc.tensor.matmul(
                        pSm[jj][:], st_op, ones_s[:], start=first, stop=last
                    )
                    for half in range(2):
                        nc.tensor.matmul(
                            pO[jj][:, half * 512 : (half + 1) * 512],
                            st_op,
                            vv[:, vti, half * 512 : (half + 1) * 512],
                            start=first,
                            stop=last,
                        )

            def own_evicts(slot, pO, pSm):
                """Evict a pass-1 slot's partials to SBUF on ACT; sums first
                so the next slot's ones-matmul (psS bank reuse) unblocks
                earliest.  Deferred by the caller until after the NEXT
                slot's first exp so that exp isn't stuck behind ~2.5us of
                evictions in the ACT FIFO."""
                for jj in range(2):
                    nc.scalar.activation(
                        sums_own[:, slot * 2 + jj : slot * 2 + jj + 1],
                        pSm[jj][:],
                        ident,
                        scale=1.0,
                    )
                for jj in range(2):
                    nc.scalar.activation(
                        oOwn[:, slot * 2 + jj, :], pO[jj][:], ident, scale=1.0
                    )

            # Pass 1: own-parity phases of slots 1,2,3 (slot 0 runs fused at
            # the end of pass 2 to keep the kernel exit path short).
            pend1 = None
            for slot in (1, 2, 3):
                Dj = DDEPTH[slot]
                pO = [psB.tile([P, 1024], f32, name="psO") for _ in range(2)]
                pSm = [psS.tile([P, 1], f32, name="psSum") for _ in range(2)]
                for i in range(Dj):
                    eS = score_tile(kT_own, slot, i, Dj, 0)
                    if i == 0 and pend1 is not None:
                        own_evicts(*pend1)
                        pend1 = None
                    av_tile(eS, pO, pSm, i, i == 0, i == Dj - 1)
                pend1 = (slot, pO, pSm)

            def v_merge(vrecv, base, c):
                """Merge one 256KB fp8 V AllGather chunk (2 k-tiles at
                vv8_peer[:, base:base+2], chunk c of vrecv) on DVE:
                peer = m0 + m1 - own, with the sum formed in a bf16 temp
                (exact for e4m3 addends) so the fp8 store is lossless.
                Reads alternate sync/gpsimd (NOT vector: a vector-queue
                read issue would sit behind combine TTs in the strict
                FIFO and couldn't fire when the collective lands)."""
                strip_reads(
                    lambda a, b: vv[a:b, 8 + base : 10 + base, :],
                    vrecv.ap()[0, :, c * 2048 : (c + 1) * 2048].rearrange(
                        "p (o s) -> p o s", s=1024
                    ),
                    [nc.gpsimd, nc.sync],
                )
                vt = mtmp.tile([P, 2, 1024], b16, name="merge_tmp")
                strip_reads(
                    lambda a, b: vt[a:b],
                    vrecv.ap()[1, :, c * 2048 : (c + 1) * 2048].rearrange(
                        "p (o s) -> p o s", s=1024
                    ),
                    [nc.sync, nc.gpsimd],
                )
                sl = vv[:, 8 + base : 10 + base, :]
                nc.vector.tensor_tensor(sl, sl, vt[:], mybir.AluOpType.add)
                nc.vector.tensor_tensor(
                    sl,
                    sl,
                    vv[:, base : base + 2, :],
                    mybir.AluOpType.subtract,
                )

            out_q = [nc.sync, nc.scalar]

            def combines_dve(slot, pO, pSm):
                """Fold own partials into the peer psum on DVE.  PSUM-freeing
                TTs lead (sums, then O adds) so the next slot's matmuls get
                their banks back earliest.  Returns the ACT half (x 1/sum
                scaling + store) as a thunk the caller defers until after
                the next slot's first exp."""
                q0 = 256 * slot
                tts, recs, o_sbs = [], [], []
                for jj in range(2):
                    tt = evss.tile([P, 1], f32, name="sum_tot")
                    nc.vector.tensor_tensor(
                        tt[:],
                        pSm[jj][:],
                        sums_own[:, slot * 2 + jj : slot * 2 + jj + 1],
                        mybir.AluOpType.add,
                    )
                    tts.append(tt)
                for jj in range(2):
                    o_sb = evsm.tile([P, 1024], f32, name="o_sb")
                    nc.vector.tensor_tensor(
                        o_sb[:],
                        pO[jj][:],
                        oOwn[:, slot * 2 + jj, :],
                        mybir.AluOpType.add,
                    )
                    o_sbs.append(o_sb)
                for jj in range(2):
                    rec = evss.tile([P, 1], f32, name="recip")
                    nc.vector.reciprocal(rec[:], tts[jj][:])
                    recs.append(rec)

                def act_half():
                    # Fused normalize+bias: o = (o_sb * 1/sum) + bvB in one
                    # scalar_tensor_tensor on DVE (Pool rejects
                    # TensorScalarPtr at codegen).  Full 4KB output rows in
                    # one DMA per q-block: halves the descriptor count so
                    # the final drain doesn't tail.
                    for jj in range(2):
                        o_sb = o_sbs[jj]
                        nc.vector.scalar_tensor_tensor(
                            o_sb[:],
                            o_sb[:],
                            recs[jj][:, 0:1],
                            bvB[:],
                            mybir.AluOpType.mult,
                            mybir.AluOpType.add,
                        )
                        out_q[jj].dma_start(
                            out_d.ap()[q0 + jj * P : q0 + (jj + 1) * P, :], o_sb[:]
                        )

                return act_half

            # All V merges before pass 2: with the pipelined collective
            # stream the V AllGathers land well before the slot-1 combines
            # are data-ready, so the merge TTs run on an idle DVE instead
            # of queueing behind (and stalling) the combines.
            v_merge(vrecv1, 0, 0)
            v_merge(vrecv1, 2, 1)
            v_merge(vrecv2, 4, 0)
            v_merge(vrecv2, 6, 1)

            # Pass 2: peer-parity phases of slots 1,2,3.
            pend2 = None
            for slot in (1, 2, 3):
                Dj = DDEPTH[slot]
                pO = [psB.tile([P, 1024], f32, name="psO") for _ in range(2)]
                pSm = [psS.tile([P, 1], f32, name="psSum") for _ in range(2)]
                for i in range(Dj):
                    eS = score_tile(kT_peer, slot, i, Dj, 1)
                    if i == 0:
                        if pend1 is not None:
                            own_evicts(*pend1)
                            pend1 = None
                        if pend2 is not None:
                            pend2()
                            pend2 = None
                    av_tile(eS, pO, pSm, 8 + i, i == 0, i == Dj - 1)
                pend2 = combines_dve(slot, pO, pSm)

            # Slot 0 (Dj=2): own + peer fused in a single accumulation --
            # normalize-only exit path on DVE in 512-col pieces, each
            # store firing as soon as its piece lands.
            pO = [psB.tile([P, 1024], f32, name="psO") for _ in range(2)]
            pSm = [psS.tile([P, 1], f32, name="psSum") for _ in range(2)]
            for phase, kTt in enumerate((kT_own, kT_peer)):
                for i in range(2):
                    eS = score_tile(kTt, 0, i, 2, phase)
                    if phase == 0 and i == 0 and pend2 is not None:
                        pend2()
                        pend2 = None
                    av_tile(
                        eS,
                        pO,
                        pSm,
                        8 * phase + i,
                        phase == 0 and i == 0,
                        phase == 1 and i == 1,
                    )
            recs0 = []
            for jj in range(2):
                rec = evss.tile([P, 1], f32, name="recip")
                nc.vector.reciprocal(rec[:], pSm[jj][:])
                recs0.append(rec)
            o_sbs0 = [evsm.tile([P, 1024], f32, name="o_sb") for _ in range(2)]
            for hf in range(2):
                for jj in range(2):
                    sl = slice(hf * 512, (hf + 1) * 512)
                    nc.vector.scalar_tensor_tensor(
                        o_sbs0[jj][:, sl],
                        pO[jj][:, sl],
                        recs0[jj][:, 0:1],
                        bvB[:, sl],
                        mybir.AluOpType.mult,
                        mybir.AluOpType.add,
                    )
                    out_q[jj].dma_start(
                        out_d.ap()[jj * P : (jj + 1) * P, hf * 512 : (hf + 1) * 512],
                        o_sbs0[jj][:, sl],
                    )

    nc.compile()
    return nc


def _get_graph():
    global _GRAPH
    if _GRAPH is None:
        _GRAPH = _build_graph()
    return _GRAPH


def _masks_for(p):
    """ADDITIVE boundary masks {0, -1e30} (pre-softmax-scale), image
    [P, 8*256] with index 8 = (slot%2)*4 + phase*2 + r.

    Slot j hosts window m = WINDOWS[p][j] with half-depth Dj; phase 0 = own
    parity (global tile 2i+p), phase 1 = peer parity (2i+1-p).  Masked
    positions are the last two tiles i in {Dj-2, Dj-1} of each phase.  The
    boundary offset 256m - 128g depends on slot only through m - Dj, which
    alternates with slot parity per core, so slots j and j+2 share masks.
    The kernel adds these to the scores psum via an identity-lhsT matmul.
    """
    masks = np.zeros((8, P, 256), dtype=np.float32)
    k_idx = np.arange(P)[:, None]
    q_idx = np.arange(256)[None, :]
    for slot in range(2):
        Dj = DDEPTH[slot]
        m = WINDOWS[p][slot]
        for phase in range(2):
            par = p if phase == 0 else 1 - p
            for r in range(2):
                i = Dj - 2 + r
                g = 2 * i + par
                valid = (g * P + k_idx) <= (256 * m + q_idx)
                masks[slot * 4 + phase * 2 + r] = np.where(valid, 0.0, -1e30)
    return np.ascontiguousarray(
        masks.astype(BF16).transpose(1, 0, 2).reshape(P, 8 * 256)
    )


def _make_in_maps(x, Wq, bq, Wk, bk, Wv, bv):
    x = np.asarray(x, dtype=np.float32)

    def wmajor(w):
        # [1024 din, 1024 out] -> [128, 2 out-half, 8 din, 512] flat image
        w = np.asarray(w, dtype=np.float32).astype(BF16)
        a = w.reshape(8, P, 2, 512)  # [din_o, p, out_half, 512]
        return np.ascontiguousarray(a.transpose(1, 2, 0, 3).reshape(P, 8192))

    def wmajor_et(w):
        # [1024 din, 1024 out] -> [128, 8 out-tile, 8 din, 128] flat image
        w = np.asarray(w, dtype=np.float32).astype(BF16)
        a = w.reshape(8, P, 8, P)  # [din_o, p, out_tile, 128]
        return np.ascontiguousarray(a.transpose(1, 2, 0, 3).reshape(P, 8192))

    def pmajor_halves(a2d):
        # [1024 d, 1024 s] -> [128, 2 s-half, 8 d_o, 512] flat image
        a = a2d.reshape(8, P, 2, 512)
        return np.ascontiguousarray(a.transpose(1, 2, 0, 3).reshape(P, 8192))

    wq_b = wmajor_et(Wq)
    wk_b = wmajor_et(Wk)
    wv_b = wmajor(Wv)
    bq2 = np.ascontiguousarray(np.asarray(bq, np.float32).reshape(8, P).T)
    bk2 = np.ascontiguousarray(np.asarray(bk, np.float32).reshape(8, P).T)
    bvb = np.ascontiguousarray(np.asarray(bv, np.float32).astype(BF16).reshape(1, D))
    ident = np.ascontiguousarray(np.eye(P, dtype=np.float32).astype(BF16))
    masks_by_p = {p: _masks_for(p) for p in (0, 1)}

    in_maps = []
    for c in range(NCORES):
        b, p = divmod(c, 2)
        xT = x[b].T.astype(BF16)
        # own-parity k-tiles {2i+p} only
        cols = np.concatenate(
            [xT[:, (2 * i + p) * P : (2 * i + p + 1) * P] for i in range(8)],
            axis=1,
        )
        xt = pmajor_halves(cols)
        qcols = np.concatenate(
            [xT[:, 256 * m : 256 * (m + 1)] for m in WINDOWS[p]], axis=1
        )
        xqt = pmajor_halves(qcols)
        in_maps.append(
            dict(
                xt=xt,
                xqt=xqt,
                wq=wq_b,
                wk=wk_b,
                wv=wv_b,
                bq=bq2,
                bk=bk2,
                bvb=bvb,
                masks=masks_by_p[p],
                ident=ident,
            )
        )
    return in_maps


def _assemble(results):
    out = np.empty((B, S, D), dtype=np.float32)
    for c in range(NCORES):
        b, p = divmod(c, 2)
        o = results[c]["out"]
        for slot, m in enumerate(WINDOWS[p]):
            out[b, 256 * m : 256 * (m + 1)] = o[256 * slot : 256 * (slot + 1)]
    return out


def _run(in_maps, trace=False, **kwargs):
    from concourse.bass_utils import run_bass_kernel_spmd

    nc = _get_graph()
    return run_bass_kernel_spmd(
        nc, in_maps, core_ids=list(range(NCORES)), trace=trace, **kwargs
    )


def kernel(x, Wq, bq, Wk, bk, Wv, bv):
    in_maps = _make_in_maps(x, Wq, bq, Wk, bk, Wv, bv)
    res = _run(in_maps)
    return _assemble(res.results)


def _install_profile_shim():
    """The agent image's ``antenv`` lacks ``axon_hooks``; recreate it so
    run_bass_kernel_spmd(trace=True) can find the NTFF profile hook, and
    stub out the artifact upload (no bucket access here)."""
    import types

    if "antenv.axon_hooks" not in sys.modules:
        mod = types.ModuleType("antenv.axon_hooks")
        mod._hook = None

        def set_axon_ntff_profile_hook(h):
            mod._hook = h

        def get_axon_ntff_profile_hook():
            return mod._hook

        mod.set_axon_ntff_profile_hook = set_axon_ntff_profile_hook
        mod.get_axon_ntff_profile_hook = get_axon_ntff_profile_hook
        sys.modules["antenv.axon_hooks"] = mod

    if sys.modules["antenv.axon_hooks"]._hook is None:
        from trn_agent_boot.trn_boot import _ntff_profile_via_ctypes

        sys.modules["antenv.axon_hooks"].set_axon_ntff_profile_hook(
            _ntff_profile_via_ctypes("/opt/axon/libaxon_pjrt.so")
        )

    from concourse import bass_utils

    bass_utils.upload_artifacts = lambda tmpdir: f"local:{tmpdir}"


def profile(inputs, **kwargs):
    """Run with tracing; returns (exec_time_ns, BassKernelResults)."""
    _install_profile_shim()
    in_maps = _make_in_maps(**inputs)
    res = _run(in_maps, trace=True, **kwargs)
    return res.exec_time_ns, res



# revision 70
# speedup vs baseline: 1.0618x; 1.0618x over previous
"""Trainium2 Bass kernel: single-head causal self-attention.

Problem: B=4, S=2048, D=1024, f32 IO.
  Q = x@Wq + bq; K = x@Wk + bk; V = x@Wv + bv
  out = softmax(causal(Q K^T / sqrt(D))) @ V

Sharding over 8 NeuronCores: core c -> (batch b = c//2, part p = c%2).
Each batch's 2048 query rows are split into eight 256-row windows
W_0..W_7 (causal depth of W_m is 2m+2 k-tiles of 128).  p=0 takes
windows [0,3,4,7], p=1 takes [1,2,5,6] -- both depth-sum to 36 so
attention work is balanced.  The SPMD graph is identical on every core:
4 window "slots" with fixed half-depths Dj=[2,4,6,8]; which global
window sits in which slot is data placement (host permutes Q rows and
output rows; boundary causal masks are inputs).

K/V are PARITY-SPLIT across the core pair: core p projects K^T and V
for global k-tiles {2j+p} only ("own", canonical positions).  The pair
exchanges halves with AllGathers -- K^T in fp8-e4m3 (2x512KB, only the
PEER half of the scores sees fp8; measured rel-err 1.266e-2 vs the
2e-2 gate), V in bf16 (2x1MB) -- and each core reconstructs the peer
half as peer = member0 + member1 - own, which is SPMD-uniform.  For
fp8 the m0+m1 sum is formed in a bf16 temp (exact for e4m3 addends) so
subtracting the fp8 own payload returns the peer member bit-exactly.

Attention is two passes: ALL own-parity phases first (slot partial O
and exp-sums evicted to SBUF via ACT, sums first -- psS bank reuse),
then all peer-parity phases combined with the own partials on DVE and
normalized+biased via one scalar_tensor_tensor (o*[1/sum] + bv; the
V-bias is NEVER projected -- normalization absorbs the sum*bv term
exactly).  Slot 0 runs last with own+peer fused in one PSUM
accumulation so the exit path is normalize-only.

Hardware model, learned the hard way (timings at ~2.0-2.2GHz P0):
- The SERIAL collective stream is the pass-2 critical path: it cannot
  start before the framework's rendezvous barrier clears (~54us) plus
  ~11us first-AG ncfw startup, runs ~12-24us/MB-per-rank (2-rank mesh,
  noisy -- 4 pairs contend), and its SDMA traffic starves DIRECT2D
  reads on all 16 rings while it runs.  Bytes are everything.
- DIRECT2D (engine-triggered) DMA costs ~200-300ns PER PARTITION-ROW
  DESCRIPTOR regardless of size, so every [128, N] bounce transfer is
  ~25us on one ring.  All bounce traffic moves as [32..64, N]
  partition strips on separate rings (strip_writes/strip_reads).
- AllGather triggers MUST be on gpsimd (walrus: DMA|Pool only).  Keep
  the gpsimd queue free of Q7 COMPUTE ops before/between triggers:
  partition_broadcast or tensor_tensor there stalls the strict FIFO
  (+20-30us).  bvB broadcast is a K=1 ones-row matmul instead.
- Engine-queue FIFO order must match semaphore-readiness order per
  queue (sync: inputs, bounce writes, bounce reads, out stores; ACT
  never carries anything that waits on a collective).
- fp8 DVE TTs get no 16-bit fast path (~2.3us per [P,2048]); keep all
  merge TTs on DVE anyway -- Q7 TTs are slower AND block the gpsimd
  read queue.
- PE runs at ~2.0-2.2GHz under all-core load (P0), ~232ns/512-col MM.

- The first krecv chunk's reads are PRE-ISSUED right after the AG-V1
  trigger so they fire the moment AG-K1 lands, inside the clean ring
  window before AG-V1's SDMA traffic starts.
- Input chunks narrower than ~2 d-tiles are a LOSS (measured +35us):
  per-partition descriptor granularity applies to input DMAs too.

Matmuls bf16 (peer-half scores fp8xbf16), softmax f32, partial-O
evictions bf16.  HW exec ~194-200us (collective-stream noise).
"""

import sys

import numpy as np

if "/opt/trn_rl_repo" not in sys.path:
    sys.path.insert(0, "/opt/trn_rl_repo")

import ml_dtypes

D = 1024
S = 2048
B = 4
P = 128
NCORES = 8
WINDOWS = {0: [0, 3, 4, 7], 1: [1, 2, 5, 6]}
DDEPTH = [2, 4, 6, 8]  # per-slot half-depth (own tiles = peer tiles = Dj)
BF16 = ml_dtypes.bfloat16

_GRAPH = None


def _build_graph():
    import concourse.bass as bass  # noqa: F401
    from concourse import bacc, mybir, tile

    f32 = mybir.dt.float32
    b16 = mybir.dt.bfloat16

    nc = bacc.Bacc(None, target_bir_lowering=False, debug=False, num_devices=NCORES)

    # Host-pre-arranged partition-major images (contiguous per partition).
    # xt: x^T columns of OWN parity k-tiles, [128, 2 s-half, 8 d_in, 512]
    # w*: [128, 2 out-half, 8 d_in, 512]
    xt_d = nc.declare_dram_parameter("xt", [P, 8192], b16, False)
    xqt_d = nc.declare_dram_parameter("xqt", [P, 8192], b16, False)
    wq_d = nc.declare_dram_parameter("wq", [P, 8192], b16, False)
    wk_d = nc.declare_dram_parameter("wk", [P, 8192], b16, False)
    wv_d = nc.declare_dram_parameter("wv", [P, 8192], b16, False)
    bq_d = nc.declare_dram_parameter("bq", [P, 8], f32, False)
    bk_d = nc.declare_dram_parameter("bk", [P, 8], f32, False)
    bvb_d = nc.declare_dram_parameter("bvb", [1, D], b16, False)
    mk_d = nc.declare_dram_parameter("masks", [P, 8 * 256], b16, False)
    id_d = nc.declare_dram_parameter("ident", [P, P], b16, False)
    out_d = nc.declare_dram_parameter("out", [1024, D], f32, True)

    f8 = mybir.dt.float8e4

    # Collective bounce buffers (internal DRAM).  The serial collective
    # stream (one ncfw; AGs never overlap) is the pass-2 critical path:
    # it can't start before the framework's rendezvous barrier clears
    # (~55us) and runs at ~17us/MB, so bytes are everything.  K^T is
    # exchanged in fp8-e4m3 (scores for the peer half only; measured
    # rel-err 1.27e-2 vs the 2e-2 gate) as TWO 512KB AllGathers -- the
    # first fires mid-K-projection.  V goes as two 1MB bf16 AGs.
    ksend1 = nc.dram_tensor("ksend1", [P, 4 * 1024], f8)
    krecv1 = nc.dram_tensor("krecv1", [2, P, 4 * 1024], f8)
    ksend2 = nc.dram_tensor("ksend2", [P, 4 * 1024], f8)
    krecv2 = nc.dram_tensor("krecv2", [2, P, 4 * 1024], f8)
    vsend1 = nc.dram_tensor("vsend1", [P, 4 * 1024], b16)
    vrecv1 = nc.dram_tensor("vrecv1", [2, P, 4 * 1024], b16)
    vsend2 = nc.dram_tensor("vsend2", [P, 4 * 1024], b16)
    vrecv2 = nc.dram_tensor("vrecv2", [2, P, 4 * 1024], b16)
    groups = [[0, 1], [2, 3], [4, 5], [6, 7]]

    with tile.TileContext(nc) as tc:
        with (
            tc.tile_pool(name="const", bufs=1) as const,
            tc.tile_pool(name="psA", bufs=2, space="PSUM") as psA,
            tc.tile_pool(name="psB", bufs=2, space="PSUM") as psB,
            tc.tile_pool(name="psS", bufs=2, space="PSUM") as psS,
            tc.tile_pool(name="evict", bufs=6) as evict,
            tc.tile_pool(name="evsm", bufs=2) as evsm,
            tc.tile_pool(name="evss", bufs=4) as evss,
            tc.tile_pool(name="mtmp", bufs=2) as mtmp,
            tc.tile_pool(name="ktmp8", bufs=2) as ktmp8,
        ):
            xt = const.tile([P, 2, 8, 512], b16, name="xt_s")
            xqt = const.tile([P, 2, 8, 512], b16, name="xqt_s")
            # wq/wk are et-major ([P, d_out tile, d_in tile, 128]) so the K
            # projection's first chunk needs only a 256KB weight slice; wv
            # stays half-major (its matmuls stream full 512-wide slices).
            w_sb = {
                n: const.tile(
                    [P, 8, 8, P] if n in ("q", "k") else [P, 2, 8, 512],
                    b16,
                    name=f"w_{n}_s",
                )
                for n in ("q", "k", "v")
            }
            kT_own = const.tile([P, 8, 1024], b16, name="kT_own")
            # fp8 shadow of kT_own: what this core SENDS (and what the
            # SPMD merge peer = m0 + m1 - own must subtract -- it has to
            # be bit-identical to the member payload).  Pass-1 keeps the
            # bf16 kT_own; only pass-2 sees fp8 K.
            kT_own8 = const.tile([P, 8, 1024], f8, name="kT_own8")
            kT_peer = const.tile([P, 8, 1024], f8, name="kT_peer")
            vv = const.tile([P, 16, D], b16, name="vv_s")
            qT = const.tile([P, 8, 1024], b16, name="qT_s")
            bq_s = const.tile([P, 8], f32, name="bq_s")
            bk_s = const.tile([P, 8], f32, name="bk_s")
            bvB = const.tile([P, D], b16, name="bvB_s")
            bvr_s = bvB[0:1, :]  # bv row loads into partition 0 of its own broadcast tile
            # Mask patterns depend only on slot parity (m - Dj alternates
            # with slot%2 per core), so 8 entries cover all 16 uses.
            mk_s = const.tile([P, 8, 256], b16, name="mk_s")
            ones_s = const.tile([P, 1], b16, name="ones_s")
            id_s = const.tile([P, P], b16, name="id_s")
            oOwn = const.tile([P, 8, 1024], b16, name="oOwn_s")
            sums_own = const.tile([P, 8], f32, name="sums_own")

            # Input DMAs: ordered so K projection unblocks first -- the
            # first chunk needs only bk + wk[et 0] (256KB) + xt half 0.
            # The K-proj-critical tensors are split across several
            # dma_starts (each lands on its own DMA ring, ~40GB/s/ring)
            # and the xt chunks issue from the scalar queue so the two
            # sequencers enqueue descriptors in parallel.
            nc.sync.dma_start(bk_s[:], bk_d.ap())
            nc.sync.dma_start(w_sb["k"][:, 0, 0:4], wk_d.ap()[:, 0:512])
            nc.sync.dma_start(w_sb["k"][:, 0, 4:8], wk_d.ap()[:, 512:1024])
            for d2 in range(4):
                nc.scalar.dma_start(
                    xt[:, 0, 2 * d2 : 2 * d2 + 2],
                    xt_d.ap()[:, d2 * 1024 : (d2 + 1) * 1024],
                )
            for d2 in range(2):
                nc.sync.dma_start(
                    xt[:, 1, 4 * d2 : 4 * d2 + 4],
                    xt_d.ap()[:, 4096 + d2 * 2048 : 4096 + (d2 + 1) * 2048],
                )
            for et in range(1, 8):
                nc.sync.dma_start(
                    w_sb["k"][:, et], wk_d.ap()[:, et * 1024 : (et + 1) * 1024]
                )
            nc.sync.dma_start(bvr_s, bvb_d.ap())
            for c in range(2):
                nc.sync.dma_start(
                    w_sb["v"][:, c], wv_d.ap()[:, c * 4096 : (c + 1) * 4096]
                )
            nc.sync.dma_start(bq_s[:], bq_d.ap())
            for et in range(8):
                nc.sync.dma_start(
                    w_sb["q"][:, et], wq_d.ap()[:, et * 1024 : (et + 1) * 1024]
                )
            for c in range(2):
                nc.sync.dma_start(xqt[:, c], xqt_d.ap()[:, c * 4096 : (c + 1) * 4096])
            nc.sync.dma_start(mk_s[:], mk_d.ap().rearrange("p (n f) -> p n f", f=256))
            nc.sync.dma_start(id_s[:], id_d.ap())
            nc.any.memset(ones_s[:], 1.0)

            # PE warm-up: junk matmuls during the initial DMA wait flip the
            # HAM clock gate before the real work starts (first ~3.4us of
            # matmul activity otherwise runs at 1.2GHz).  Sized so the
            # queue drains right as the split xt/wk chunks land (~12us);
            # extra warm-ups would delay K-proj in the in-order PE FIFO.
            warm_in = const.tile([P, 512], b16, name="warm_in")
            nc.vector.memset(warm_in[:], 0.0)
            # Tiny TT warms the Q7 tensor-op ucode library so the first
            # real gpsimd merge op doesn't pay a LIBRARY_RELOAD mid-
            # exchange (the reload itself is cheap, but it must happen
            # before the strict-FIFO queue fills with gated DMAs).
            nc.gpsimd.tensor_tensor(
                warm_in[:, 0:1], ones_s[:], ones_s[:], mybir.AluOpType.add
            )
            for _ in range(2):
                wps = psA.tile([P, 512], f32, name="ps_mm")
                for _ in range(4):
                    nc.tensor.matmul(wps[:1], ones_s[:], warm_in[:], start=True, stop=True)

            ident = mybir.ActivationFunctionType.Identity
            ones_r = const.tile([1, P], b16, name="ones_r")
            nc.any.memset(ones_r[:], 1.0)

            def strip_writes(dst_dram, src_fn, queues):
                """Bounce-buffer write as 4 partition strips.  A DIRECT2D
                descriptor covers ONE partition's contiguous run (~185ns
                each regardless of size), so a [128, N] write costs ~24us
                on one ring no matter what N is.  [32, N] strips on 4
                rings move the same data ~4x faster."""
                for s in range(4):
                    queues[s % len(queues)].dma_start(
                        dst_dram.ap()[32 * s : 32 * (s + 1), :],
                        src_fn(32 * s, 32 * (s + 1)),
                    )

            def strip_reads(dst_fn, src_ap, queues, nstrip=2):
                """Post-collective DRAM->SBUF reads as partition strips
                (same per-partition-descriptor economics as the writes)."""
                step = P // nstrip
                for s in range(nstrip):
                    queues[s % len(queues)].dma_start(
                        dst_fn(step * s, step * (s + 1)),
                        src_ap[step * s : step * (s + 1)],
                    )

            # K^T projection (own-parity 1024 keys):
            # psum[d_out 128, s 512] = sum_di Wk[di,e].T @ xT[di,s]
            for et in range(8):
                ksend = ksend1 if et < 4 else ksend2
                for sw in range(2):
                    ps = psA.tile([P, 512], f32, name="ps_mm")
                    for di in range(8):
                        nc.tensor.matmul(
                            ps[:],
                            w_sb["k"][:, et, di, :],
                            xt[:, sw, di, :],
                            start=(di == 0),
                            stop=(di == 7),
                        )
                    nc.scalar.activation(
                        kT_own[:, et, sw * 512 : (sw + 1) * 512],
                        ps[:],
                        ident,
                        bias=bk_s[:, et : et + 1],
                        scale=1.0,
                    )
                    nc.scalar.activation(
                        kT_own8[:, et, sw * 512 : (sw + 1) * 512],
                        ps[:],
                        ident,
                        bias=bk_s[:, et : et + 1],
                        scale=1.0,
                    )
                if et == 3 or et == 7:
                    strip_writes(
                        ksend,
                        lambda a, b, lo=et - 3: kT_own8[a:b, lo : lo + 4, :],
                        [nc.sync, nc.scalar, nc.gpsimd, nc.sync],
                    )
                    nc.gpsimd.collective_compute(
                        "AllGather",
                        mybir.AluOpType.bypass,
                        replica_groups=groups,
                        ins=[(ksend1 if et == 3 else ksend2).ap().opt()],
                        outs=[(krecv1 if et == 3 else krecv2).ap().opt()],
                    )

            # V projection: OWN-parity 8 k-tiles only; the peer half
            # arrives via a single 2MB AllGather (collective cost is
            # roughly per-op, not per-byte, at these sizes).
            for st in range(8):
                for dw in range(2):
                    ps = psA.tile([P, 512], f32, name="ps_mm")
                    for di in range(8):
                        nc.tensor.matmul(
                            ps[:],
                            xt[:, st // 4, di, (st % 4) * P : (st % 4 + 1) * P],
                            w_sb["v"][:, dw, di, :],
                            start=(di == 0),
                            stop=(di == 7),
                        )
                    nc.scalar.activation(
                        vv[:, st, dw * 512 : (dw + 1) * 512], ps[:], ident, scale=1.0
                    )
                if st in (3, 7):
                    vsend, vrecv, lo = {
                        3: (vsend1, vrecv1, 0),
                        7: (vsend2, vrecv2, 4),
                    }[st]
                    strip_writes(
                        vsend,
                        lambda a, b, lo=lo: vv[a:b, lo : lo + 4, :],
                        [nc.sync, nc.scalar, nc.gpsimd, nc.sync],
                    )
                    nc.gpsimd.collective_compute(
                        "AllGather",
                        mybir.AluOpType.bypass,
                        replica_groups=groups,
                        ins=[vsend.ap().opt()],
                        outs=[vrecv.ap().opt()],
                    )
                if st == 3:
                    # Pre-issue the first krecv chunk's reads HERE (right
                    # after the AG-V1 trigger in the sync/gpsimd FIFOs):
                    # emitted any later they queue behind the vsend2
                    # writes and can only fire ~86us; from here they fire
                    # the moment AG-K1 lands (~79us), inside the clean
                    # ring window before AG-V1's SDMA traffic starts.
                    ek8 = ktmp8.tile([P, 2, 1024], f8, name="merge_f8")
                    strip_reads(
                        lambda a, b: ek8[a:b],
                        krecv1.ap()[0, :, 0:2048].rearrange(
                            "p (o s) -> p o s", s=1024
                        ),
                        [nc.gpsimd, nc.sync],
                    )
                    ekt = ktmp8.tile([P, 2, 1024], f8, name="merge_f8")
                    strip_reads(
                        lambda a, b: ekt[a:b],
                        krecv1.ap()[1, :, 0:2048].rearrange(
                            "p (o s) -> p o s", s=1024
                        ),
                        [nc.sync, nc.gpsimd],
                    )

            # Q^T projection (this core's 1024 query rows)
            for et in range(8):
                for sw in range(2):
                    ps = psA.tile([P, 512], f32, name="ps_mm")
                    for di in range(8):
                        nc.tensor.matmul(
                            ps[:],
                            w_sb["q"][:, et, di, :],
                            xqt[:, sw, di, :],
                            start=(di == 0),
                            stop=(di == 7),
                        )
                    nc.scalar.activation(
                        qT[:, et, sw * 512 : (sw + 1) * 512],
                        ps[:],
                        ident,
                        bias=bq_s[:, et : et + 1],
                        scale=1.0,
                    )

            # V-bias never rides the projection: out = (eS @ V0)/sum + bv
            # exactly (the normalization absorbs the sum*bv term), so bv
            # is broadcast once to all partitions here (K=1 ones-row
            # matmul -- NOT gpsimd.partition_broadcast, whose Q7 ucode
            # LIBRARY_RELOAD would block the gpsimd queue, and with it
            # the AllGather triggers, for ~30us; emitted after the
            # projections so it never delays them in the in-order PE
            # FIFO) and added at the normalize step.
            for half in range(2):
                ps = psA.tile([P, 512], f32, name="ps_mm")
                nc.tensor.matmul(
                    ps[:],
                    ones_r[0:1, :],
                    bvr_s[:, half * 512 : (half + 1) * 512],
                    start=True,
                    stop=True,
                )
                nc.scalar.activation(
                    bvB[:, half * 512 : (half + 1) * 512], ps[:], ident, scale=1.0
                )

            # K merge on DVE: peer = member0 + member1 - own (SPMD-uniform).
            # The DVE queue has no other work between the projections and
            # the pass-2 combines, so a late AllGather can only delay the
            # merge itself, never an unrelated op queued behind it.
            # Reads issue from gpsimd and sync AFTER their bounce-write
            # strips (queue order matches semaphore readiness order, so
            # no FIFO inversion can strand a later-ready transfer).
            # fp8 wrinkle: m0 + m1 must be formed in a bf16 temp (exact
            # for e4m3 addends); subtracting the fp8 own payload then
            # yields the peer member's exact e4m3 value, so the final fp8
            # store is lossless.
            for h, krecv in enumerate((krecv1, krecv2)):
                for c in range(2):
                    o0 = 4 * h + 2 * c
                    if h == 0 and c == 0:
                        k8, kt = ek8, ekt  # pre-issued at st==3 above
                    else:
                        k8 = ktmp8.tile([P, 2, 1024], f8, name="merge_f8")
                        strip_reads(
                            lambda a, b, k8=k8: k8[a:b],
                            krecv.ap()[0, :, c * 2048 : (c + 1) * 2048].rearrange(
                                "p (o s) -> p o s", s=1024
                            ),
                            [nc.gpsimd, nc.sync],
                        )
                        kt = ktmp8.tile([P, 2, 1024], f8, name="merge_f8")
                        strip_reads(
                            lambda a, b, kt=kt: kt[a:b],
                            krecv.ap()[1, :, c * 2048 : (c + 1) * 2048].rearrange(
                                "p (o s) -> p o s", s=1024
                            ),
                            [nc.sync, nc.gpsimd],
                        )
                    # fp8 TTs get no 16-bit fast path (~2.3us per op).
                    # Both halves stay on DVE: Q7 tensor ops measured even
                    # slower AND strict-FIFO-block the v_merge strip reads
                    # queued behind them on gpsimd (+29us end-to-end).
                    eng = nc.vector
                    tb = mtmp.tile([P, 2, 1024], b16, name="merge_tmp")
                    eng.tensor_tensor(tb[:], k8[:], kt[:], mybir.AluOpType.add)
                    eng.tensor_tensor(
                        kT_peer[:, o0 : o0 + 2, :],
                        tb[:],
                        kT_own8[:, o0 : o0 + 2, :],
                        mybir.AluOpType.subtract,
                    )

            # Attention pass 1: OWN-parity k-tiles for every slot (deepest
            # first).  scores^T [k 128, q 256] per tile; exp on ACT (f32
            # psum -> bf16); boundary masks on GPSIMD for the last 2 tiles;
            # O_unnorm[q,d] += expS^T.T @ V; sums via ones-matmul.  Partial
            # O and sums evict to SBUF (ACT) to free PSUM for the next slot.
            inv_sqrt_d = float(1.0 / np.sqrt(D))
            exp_f = mybir.ActivationFunctionType.Exp

            def score_tile(kTt, slot, i, Dj, phase):
                """scores^T psum for k-tile i of a slot's phase.  Boundary
                tiles get the additive -1e30 mask as a 9th accumulation
                matmul (identity lhsT) -- keeps the mask on the PE so the
                exp -> AV chain never waits on another engine."""
                q0 = 256 * slot
                masked = i >= Dj - 2
                ps = psA.tile([P, 256], f32, name="ps_mm")
                for di in range(8):
                    nc.tensor.matmul(
                        ps[:, :256],
                        kTt[:, di, i * P : (i + 1) * P],
                        qT[:, di, q0 : q0 + 256],
                        start=(di == 0),
                        stop=(di == 7) and not masked,
                    )
                if masked:
                    nc.tensor.matmul(
                        ps[:, :256],
                        id_s[:],
                        mk_s[:, (slot % 2) * 4 + phase * 2 + (i - (Dj - 2)), :],
                        start=False,
                        stop=True,
                    )
                eS = evict.tile([P, 256], b16, name="eS")
                nc.scalar.activation(eS[:], ps[:, :256], exp_f, scale=inv_sqrt_d)
                return eS

            def av_tile(eS, pO, pSm, vti, first, last):
                # pO[jj] is a 2-bank [P, 1024] psum tile; each matmul still
                # targets a single bank via the half slice.
                for jj in range(2):
                    st_op = eS[:, jj * P : (jj + 1) * P]
                    nc.tensor.matmul(
                        pSm[jj][:], st_op, ones_s[:], start=first, stop=last
                    )
                    for half in range(2):
                        nc.tensor.matmul(
                            pO[jj][:, half * 512 : (half + 1) * 512],
                            st_op,
                            vv[:, vti, half * 512 : (half + 1) * 512],
                            start=first,
                            stop=last,
                        )

            def own_evicts(slot, pO, pSm):
                """Evict a pass-1 slot's partials to SBUF on ACT; sums first
                so the next slot's ones-matmul (psS bank reuse) unblocks
                earliest.  Deferred by the caller until after the NEXT
                slot's first exp so that exp isn't stuck behind ~2.5us of
                evictions in the ACT FIFO."""
                for jj in range(2):
                    nc.scalar.activation(
                        sums_own[:, slot * 2 + jj : slot * 2 + jj + 1],
                        pSm[jj][:],
                        ident,
                        scale=1.0,
                    )
                for jj in range(2):
                    nc.scalar.activation(
                        oOwn[:, slot * 2 + jj, :], pO[jj][:], ident, scale=1.0
                    )

            # Pass 1: own-parity phases of slots 1,2,3 (slot 0 runs fused at
            # the end of pass 2 to keep the kernel exit path short).
            pend1 = None
            for slot in (1, 2, 3):
                Dj = DDEPTH[slot]
                pO = [psB.tile([P, 1024], f32, name="psO") for _ in range(2)]
                pSm = [psS.tile([P, 1], f32, name="psSum") for _ in range(2)]
                for i in range(Dj):
                    eS = score_tile(kT_own, slot, i, Dj, 0)
                    if i == 0 and pend1 is not None:
                        own_evicts(*pend1)
                        pend1 = None
                    av_tile(eS, pO, pSm, i, i == 0, i == Dj - 1)
                pend1 = (slot, pO, pSm)

            def v_merge(vrecv, base, c):
                """Merge one 256KB fp8 V AllGather chunk (2 k-tiles at
                vv8_peer[:, base:base+2], chunk c of vrecv) on DVE:
                peer = m0 + m1 - own, with the sum formed in a bf16 temp
                (exact for e4m3 addends) so the fp8 store is lossless.
                Reads alternate sync/gpsimd (NOT vector: a vector-queue
                read issue would sit behind combine TTs in the strict
                FIFO and couldn't fire when the collective lands)."""
                strip_reads(
                    lambda a, b: vv[a:b, 8 + base : 10 + base, :],
                    vrecv.ap()[0, :, c * 2048 : (c + 1) * 2048].rearrange(
                        "p (o s) -> p o s", s=1024
                    ),
                    [nc.gpsimd, nc.sync],
                )
                vt = mtmp.tile([P, 2, 1024], b16, name="merge_tmp")
                strip_reads(
                    lambda a, b: vt[a:b],
                    vrecv.ap()[1, :, c * 2048 : (c + 1) * 2048].rearrange(
                        "p (o s) -> p o s", s=1024
                    ),
                    [nc.sync, nc.gpsimd],
                )
                sl = vv[:, 8 + base : 10 + base, :]
                nc.vector.tensor_tensor(sl, sl, vt[:], mybir.AluOpType.add)
                nc.vector.tensor_tensor(
                    sl,
                    sl,
                    vv[:, base : base + 2, :],
                    mybir.AluOpType.subtract,
                )

            out_q = [nc.sync, nc.scalar]

            def combines_dve(slot, pO, pSm):
                """Fold own partials into the peer psum on DVE.  PSUM-freeing
                TTs lead (sums, then O adds) so the next slot's matmuls get
                their banks back earliest.  Returns the ACT half (x 1/sum
                scaling + store) as a thunk the caller defers until after
                the next slot's first exp."""
                q0 = 256 * slot
                tts, recs, o_sbs = [], [], []
                for jj in range(2):
                    tt = evss.tile([P, 1], f32, name="sum_tot")
                    nc.vector.tensor_tensor(
                        tt[:],
                        pSm[jj][:],
                        sums_own[:, slot * 2 + jj : slot * 2 + jj + 1],
                        mybir.AluOpType.add,
                    )
                    tts.append(tt)
                for jj in range(2):
                    o_sb = evsm.tile([P, 1024], f32, name="o_sb")
                    nc.vector.tensor_tensor(
                        o_sb[:],
                        pO[jj][:],
                        oOwn[:, slot * 2 + jj, :],
                        mybir.AluOpType.add,
                    )
                    o_sbs.append(o_sb)
                for jj in range(2):
                    rec = evss.tile([P, 1], f32, name="recip")
                    nc.vector.reciprocal(rec[:], tts[jj][:])
                    recs.append(rec)

                def act_half():
                    # Fused normalize+bias: o = (o_sb * 1/sum) + bvB in one
                    # scalar_tensor_tensor on DVE (Pool rejects
                    # TensorScalarPtr at codegen).  Full 4KB output rows in
                    # one DMA per q-block: halves the descriptor count so
                    # the final drain doesn't tail.
                    for jj in range(2):
                        o_sb = o_sbs[jj]
                        nc.vector.scalar_tensor_tensor(
                            o_sb[:],
                            o_sb[:],
                            recs[jj][:, 0:1],
                            bvB[:],
                            mybir.AluOpType.mult,
                            mybir.AluOpType.add,
                        )
                        out_q[jj].dma_start(
                            out_d.ap()[q0 + jj * P : q0 + (jj + 1) * P, :], o_sb[:]
                        )

                return act_half

            # All V merges before pass 2: with the pipelined collective
            # stream the V AllGathers land well before the slot-1 combines
            # are data-ready, so the merge TTs run on an idle DVE instead
            # of queueing behind (and stalling) the combines.
            v_merge(vrecv1, 0, 0)
            v_merge(vrecv1, 2, 1)
            v_merge(vrecv2, 4, 0)
            v_merge(vrecv2, 6, 1)

            # Pass 2: peer-parity phases of slots 1,2,3.
            pend2 = None
            for slot in (1, 2, 3):
                Dj = DDEPTH[slot]
                pO = [psB.tile([P, 1024], f32, name="psO") for _ in range(2)]
                pSm = [psS.tile([P, 1], f32, name="psSum") for _ in range(2)]
                for i in range(Dj):
                    eS = score_tile(kT_peer, slot, i, Dj, 1)
                    if i == 0:
                        if pend1 is not None:
                            own_evicts(*pend1)
                            pend1 = None
                        if pend2 is not None:
                            pend2()
                            pend2 = None
                    av_tile(eS, pO, pSm, 8 + i, i == 0, i == Dj - 1)
                pend2 = combines_dve(slot, pO, pSm)

            # Slot 0 (Dj=2): own + peer fused in a single accumulation --
            # normalize-only exit path on DVE in 512-col pieces, each
            # store firing as soon as its piece lands.
            pO = [psB.tile([P, 1024], f32, name="psO") for _ in range(2)]
            pSm = [psS.tile([P, 1], f32, name="psSum") for _ in range(2)]
            for phase, kTt in enumerate((kT_own, kT_peer)):
                for i in range(2):
                    eS = score_tile(kTt, 0, i, 2, phase)
                    if phase == 0 and i == 0 and pend2 is not None:
                        pend2()
                        pend2 = None
                    av_tile(
                        eS,
                        pO,
                        pSm,
                        8 * phase + i,
                        phase == 0 and i == 0,
                        phase == 1 and i == 1,
                    )
            recs0 = []
            for jj in range(2):
                rec = evss.tile([P, 1], f32, name="recip")
                nc.vector.reciprocal(rec[:], pSm[jj][:])
                recs0.append(rec)
            o_sbs0 = [evsm.tile([P, 1024], f32, name="o_sb") for _ in range(2)]
            for hf in range(2):
                for jj in range(2):
                    sl = slice(hf * 512, (hf + 1) * 512)
                    nc.vector.scalar_tensor_tensor(
                        o_sbs0[jj][:, sl],
                        pO[jj][:, sl],
                        recs0[jj][:, 0:1],
                        bvB[:, sl],
                        mybir.AluOpType.mult,
                        mybir.AluOpType.add,
                    )
                    out_q[jj].dma_start(
                        out_d.ap()[jj * P : (jj + 1) * P, hf * 512 : (hf + 1) * 512],
                        o_sbs0[jj][:, sl],
                    )

    nc.compile()
    return nc


def _get_graph():
    global _GRAPH
    if _GRAPH is None:
        _GRAPH = _build_graph()
    return _GRAPH


def _masks_for(p):
    """ADDITIVE boundary masks {0, -1e30} (pre-softmax-scale), image
    [P, 8*256] with index 8 = (slot%2)*4 + phase*2 + r.

    Slot j hosts window m = WINDOWS[p][j] with half-depth Dj; phase 0 = own
    parity (global tile 2i+p), phase 1 = peer parity (2i+1-p).  Masked
    positions are the last two tiles i in {Dj-2, Dj-1} of each phase.  The
    boundary offset 256m - 128g depends on slot only through m - Dj, which
    alternates with slot parity per core, so slots j and j+2 share masks.
    The kernel adds these to the scores psum via an identity-lhsT matmul.
    """
    masks = np.zeros((8, P, 256), dtype=np.float32)
    k_idx = np.arange(P)[:, None]
    q_idx = np.arange(256)[None, :]
    for slot in range(2):
        Dj = DDEPTH[slot]
        m = WINDOWS[p][slot]
        for phase in range(2):
            par = p if phase == 0 else 1 - p
            for r in range(2):
                i = Dj - 2 + r
                g = 2 * i + par
                valid = (g * P + k_idx) <= (256 * m + q_idx)
                masks[slot * 4 + phase * 2 + r] = np.where(valid, 0.0, -1e30)
    return np.ascontiguousarray(
        masks.astype(BF16).transpose(1, 0, 2).reshape(P, 8 * 256)
    )


def _make_in_maps(x, Wq, bq, Wk, bk, Wv, bv):
    x = np.asarray(x, dtype=np.float32)

    def wmajor(w):
        # [1024 din, 1024 out] -> [128, 2 out-half, 8 din, 512] flat image
        w = np.asarray(w, dtype=np.float32).astype(BF16)
        a = w.reshape(8, P, 2, 512)  # [din_o, p, out_half, 512]
        return np.ascontiguousarray(a.transpose(1, 2, 0, 3).reshape(P, 8192))

    def wmajor_et(w):
        # [1024 din, 1024 out] -> [128, 8 out-tile, 8 din, 128] flat image
        w = np.asarray(w, dtype=np.float32).astype(BF16)
        a = w.reshape(8, P, 8, P)  # [din_o, p, out_tile, 128]
        return np.ascontiguousarray(a.transpose(1, 2, 0, 3).reshape(P, 8192))

    def pmajor_halves(a2d):
        # [1024 d, 1024 s] -> [128, 2 s-half, 8 d_o, 512] flat image
        a = a2d.reshape(8, P, 2, 512)
        return np.ascontiguousarray(a.transpose(1, 2, 0, 3).reshape(P, 8192))

    wq_b = wmajor_et(Wq)
    wk_b = wmajor_et(Wk)
    wv_b = wmajor(Wv)
    bq2 = np.ascontiguousarray(np.asarray(bq, np.float32).reshape(8, P).T)
    bk2 = np.ascontiguousarray(np.asarray(bk, np.float32).reshape(8, P).T)
    bvb = np.ascontiguousarray(np.asarray(bv, np.float32).astype(BF16).reshape(1, D))
    ident = np.ascontiguousarray(np.eye(P, dtype=np.float32).astype(BF16))
    masks_by_p = {p: _masks_for(p) for p in (0, 1)}

    in_maps = []
    for c in range(NCORES):
        b, p = divmod(c, 2)
        xT = x[b].T.astype(BF16)
        # own-parity k-tiles {2i+p} only
        cols = np.concatenate(
            [xT[:, (2 * i + p) * P : (2 * i + p + 1) * P] for i in range(8)],
            axis=1,
        )
        xt = pmajor_halves(cols)
        qcols = np.concatenate(
            [xT[:, 256 * m : 256 * (m + 1)] for m in WINDOWS[p]], axis=1
        )
        xqt = pmajor_halves(qcols)
        in_maps.append(
            dict(
                xt=xt,
                xqt=xqt,
                wq=wq_b,
                wk=wk_b,
                wv=wv_b,
                bq=bq2,
                bk=bk2,
                bvb=bvb,
                masks=masks_by_p[p],
                ident=ident,
            )
        )
    return in_maps


def _assemble(results):
    out = np.empty((B, S, D), dtype=np.float32)
    for c in range(NCORES):
        b, p = divmod(c, 2)
        o = results[c]["out"]
        for slot, m in enumerate(WINDOWS[p]):
            out[b, 256 * m : 256 * (m + 1)] = o[256 * slot : 256 * (slot + 1)]
    return out


def _run(in_maps, trace=False, **kwargs):
    from concourse.bass_utils import run_bass_kernel_spmd

    nc = _get_graph()
    return run_bass_kernel_spmd(
        nc, in_maps, core_ids=list(range(NCORES)), trace=trace, **kwargs
    )


def kernel(x, Wq, bq, Wk, bk, Wv, bv):
    in_maps = _make_in_maps(x, Wq, bq, Wk, bk, Wv, bv)
    res = _run(in_maps)
    return _assemble(res.results)


def _install_profile_shim():
    """The agent image's ``antenv`` lacks ``axon_hooks``; recreate it so
    run_bass_kernel_spmd(trace=True) can find the NTFF profile hook, and
    stub out the artifact upload (no bucket access here)."""
    import types

    if "antenv.axon_hooks" not in sys.modules:
        mod = types.ModuleType("antenv.axon_hooks")
        mod._hook = None

        def set_axon_ntff_profile_hook(h):
            mod._hook = h

        def get_axon_ntff_profile_hook():
            return mod._hook

        mod.set_axon_ntff_profile_hook = set_axon_ntff_profile_hook
        mod.get_axon_ntff_profile_hook = get_axon_ntff_profile_hook
        sys.modules["antenv.axon_hooks"] = mod

    if sys.modules["antenv.axon_hooks"]._hook is None:
        from trn_agent_boot.trn_boot import _ntff_profile_via_ctypes

        sys.modules["antenv.axon_hooks"].set_axon_ntff_profile_hook(
            _ntff_profile_via_ctypes("/opt/axon/libaxon_pjrt.so")
        )

    from concourse import bass_utils

    bass_utils.upload_artifacts = lambda tmpdir: f"local:{tmpdir}"


def profile(inputs, **kwargs):
    """Run with tracing; returns (exec_time_ns, BassKernelResults)."""
    _install_profile_shim()
    in_maps = _make_in_maps(**inputs)
    res = _run(in_maps, trace=True, **kwargs)
    return res.exec_time_ns, res

